# revision 28
# baseline (speedup 1.0000x reference)
"""Trainium2 Bass kernel for nn_ConsistLoss (retrieval_knn).

Math notes
----------
reference() = mean(|rigid_refine - pred^T|) where
  rigid_refine = rigid_recon - mean_i(laplace_x_i - laplace_y_i)
  laplace_c_i  = (sum_{j in 6NN_c(i)} c_j - 6*q_i) / 5       (c in {x=rigid_recon, y})
The -6*q_i terms cancel in (laplace_x - laplace_y), and only the MEAN over all
i is needed, so:
  mean_vec = ( sum_j cx(j)*x_j - sum_j cy(j)*y_j ) / (5*N)
where cx(j) = #queries having ref j among their 6 nearest (mask column sums).

Device work per core (512 queries x 4096 refs x 2 clouds):
  s[q,j] = 2*q.x_j - |x_j|^2  (row-constant |q|^2 shift is rank-safe), via
  fp8e4m3 DoubleRow matmuls: 14 split-product rows in 7 partition pairs,
  0.5 cyc/col.  PSUM f32 -> SBUF fp16 copies (ACT 3 / DVE 1 per query tile),
  threshold via pairwise tensor-max folds 4096->512 (fold1 DVE, fold2/3
  GpSimd) + MAX8.  Masks: refs [0,3072) is_ge(s16,t6) -> bf16 on DVE (4x);
  refs [3072,4096) Sign(s16-mid) -> fp8 +-1 on ACT.  Column sums on PE
  incrementally per query tile (bf16 ones-matmul; fp8 DoubleRow contracts 2
  query tiles per matmul), accumulated in PSUM at 4 partition offsets so one
  [4,512] copy drains 4 chunk-sums.  Host: Kabsch (3x3 SVD) + O(N) decode.
"""

import os
from contextlib import ExitStack

import numpy as np

import concourse.bass as bass  # noqa: F401  (AP types / plumbing)
import concourse.tile as tile
from concourse import bacc, mybir
from concourse.bass_utils import run_bass_kernel_spmd

N = 4096          # points per cloud
NCORES = 8
NQ = N // NCORES  # 512 queries per core
P = 128           # SBUF partitions
QT = NQ // P      # 4 query tiles per core
CHS = 512         # fp32 PSUM bank = 512 f32
KP = 7            # fp8 split rows: 14 = 7 partitions x 2 DoubleRow pairs
NBF = 2048        # refs [0, NBF) -> bf16 masks, rest fp8 (DR colsum)
L_K = 6

_cache = {}
last_results = None  # test harness reads exec_time_ns off this


def _build_bass():
    nc = bacc.Bacc(
        "TRN2", target_bir_lowering=False, debug=False, num_devices=NCORES
    )
    f32 = mybir.dt.float32
    f16 = mybir.dt.float16
    bf16 = mybir.dt.bfloat16
    fp8 = mybir.dt.float8e4
    DR = mybir.MatmulPerfMode.DoubleRow
    NF8 = N - NBF

    qa_d = nc.dram_tensor("qa", [2 * KP, NQ], fp8, kind="ExternalInput")
    rx_d = nc.dram_tensor("rx", [2 * KP, N], fp8, kind="ExternalInput")
    ry_d = nc.dram_tensor("ry", [2 * KP, N], fp8, kind="ExternalInput")
    cnt_d = nc.dram_tensor("cnt", [1, 2 * N], f32, kind="ExternalOutput")

    with ExitStack() as ctx:
        tc = ctx.enter_context(tile.TileContext(nc))
        const_pool = ctx.enter_context(tc.tile_pool(name="const", bufs=1))
        s_pool = ctx.enter_context(tc.tile_pool(name="s16", bufs=3))
        f_pool = ctx.enter_context(tc.tile_pool(name="fold", bufs=3))
        t_pool = ctx.enter_context(tc.tile_pool(name="t8", bufs=3))
        mb_pool = ctx.enter_context(tc.tile_pool(name="mb", bufs=6))
        mf_pool = ctx.enter_context(tc.tile_pool(name="mf", bufs=4))
        ps_pool = ctx.enter_context(tc.tile_pool(name="ps", bufs=2, space="PSUM"))
        cp_pool = ctx.enter_context(tc.tile_pool(name="cp", bufs=2, space="PSUM"))

        qa = const_pool.tile([2 * KP, NQ], fp8)
        nc.sync.dma_start(qa[:], qa_d.ap())
        rx = const_pool.tile([2 * KP, N], fp8)
        nc.sync.dma_start(rx[:], rx_d.ap())
        ry = const_pool.tile([2 * KP, N], fp8)
        nc.sync.dma_start(ry[:], ry_d.ap())
        # DoubleRow ldweights needs the pair-dim step 16B-aligned -> pad
        ones8 = const_pool.tile([P, 2, 16], fp8)
        nc.vector.memset(ones8[:], 1.0)
        onesb = const_pool.tile([P, 1], bf16)
        nc.vector.memset(onesb[:], 1.0)
        out_sb = const_pool.tile([1, 2 * N], f32)
        # PE warm-up: ~reach HAM K=8/8 while the rx/ry input DMAs stream in.
        # Back-to-back matmuls on a scratch bank for >3us of continuous busy.
        warm = const_pool.tile([P, CHS], bf16)
        nc.vector.memset(warm[:], 0.0)
        wps = cp_pool.tile([1, 2 * CHS], f32, tag="cp", name="wps")
        for _ in range(16):
            nc.tensor.matmul(wps[:, :CHS], onesb[:], warm[:], start=True, stop=True)

        # --- software-pipelined main loop ------------------------------
        # iteration T: PE scores(T), ACT copies(T); DVE threshold+masks(T-1);
        # PE colsum chunks as clouds complete.  Global tile index T = 4*ci+qt.
        NT = 2 * QT
        state = {}      # T -> (s16, mb, mf, t8)
        mfs = {}        # ci -> [pair0, pair1]
        pend = []       # colsum chunk-sum work items (ci, ch)
        ndrain = [0]

        def emit_scores(T):
            ci, qt = divmod(T, QT)
            r = rx if ci == 0 else ry  # [2*KP, N] fp8
            s16 = s_pool.tile([P, N], f16, tag="s16", name=f"s16_{T}")
            for h in range(4):  # 4 PSUM tiles of [P, 1024] (2 banks each)
                ps = ps_pool.tile([P, 2 * CHS], f32, tag="ps", name=f"ps{T}_{h}")
                for k in range(2):
                    nc.tensor.matmul(
                        ps[:, k * CHS : (k + 1) * CHS],
                        qa[:, qt * P : (qt + 1) * P],
                        r[:, (2 * h + k) * CHS : (2 * h + k + 1) * CHS],
                        start=True,
                        stop=True,
                    )
                nc.scalar.copy(s16[:, h * 2 * CHS : (h + 1) * 2 * CHS], ps[:])
            state[T] = s16

        def emit_threshold_masks(T):
            ci, qt = divmod(T, QT)
            s16 = state[T]
            f1 = f_pool.tile([P, N // 2], f16, tag="f1", name=f"f1_{T}")
            nc.vector.tensor_max(f1[:], s16[:, : N // 2], s16[:, N // 2 :])
            f2 = f_pool.tile([P, N // 4], f16, tag="f2", name=f"f2_{T}")
            nc.vector.tensor_max(f2[:], f1[:, : N // 4], f1[:, N // 4 :])
            f3 = f_pool.tile([P, N // 8], f16, tag="f3", name=f"f3_{T}")
            nc.vector.tensor_max(f3[:], f2[:, : N // 8], f2[:, N // 8 :])
            t8 = t_pool.tile([P, 8], f32, tag="t8", name=f"t8_{T}")
            nc.vector.max(t8[:], f3[:])
            # masks vs t6 (exact top-6 incl. fp16 ties):
            # refs [0, NBF): 0/1 bf16 (4x); refs [NBF, N): 0/1 fp8 (2x)
            mb = mb_pool.tile([P, NBF], bf16, tag="mb", name=f"mb_{T}")
            nc.vector.tensor_scalar(
                mb[:], s16[:, :NBF], t8[:, 5:6], None, mybir.AluOpType.is_ge
            )
            if qt % 2 == 0:
                mfs.setdefault(ci, []).append(
                    mf_pool.tile([P, 2, NF8], fp8, tag="mf", name=f"mf_{T}")
                )
            mf = mfs[ci][qt // 2]
            qi = qt % 2
            nc.vector.tensor_scalar(
                mf[:, qi : qi + 1, :],
                s16[:, NBF:N],
                t8[:, 5:6],
                None,
                mybir.AluOpType.is_ge,
            )
            state[T] = (s16, mb)
            if qt == QT - 1:
                pend.extend((ci, ch) for ch in range(N // CHS))

        mbq = {}  # (ci, qt) -> mb tile

        def emit_colsum(budget):
            # emit up to `budget` chunk-sums; drain cp tiles pairwise
            k = 0
            while pend and (budget is None or k < budget):
                ci, ch = pend.pop(0)
                t16 = ci * 8 + ch
                sub = t16 % 2
                if sub == 0:
                    cps = cp_pool.tile(
                        [1, 2 * CHS], f32, tag="cp", name=f"cp_{t16}"
                    )
                    state[("cp", ci, ch // 2)] = cps
                else:
                    cps = state[("cp", ci, ch // 2)]
                dst = cps[:, sub * CHS : (sub + 1) * CHS]
                if ch < NBF // CHS:
                    for qt in range(QT):
                        nc.tensor.matmul(
                            dst,
                            onesb[:],
                            mbq[(ci, qt)][:, ch * CHS : (ch + 1) * CHS],
                            start=(qt == 0),
                            stop=(qt == QT - 1),
                        )
                else:
                    chf = ch - NBF // CHS
                    for pi in range(2):
                        nc.tensor.matmul(
                            dst,
                            ones8[:, :, 0:1],
                            mfs[ci][pi][:, :, chf * CHS : (chf + 1) * CHS],
                            start=(pi == 0),
                            stop=(pi == 1),
                            perf_mode=DR,
                        )
                if sub == 1:
                    col = ci * N + (ch - 1) * CHS
                    dstc = out_sb[0:1, col : col + 2 * CHS]
                    if ndrain[0] % 2 == 0:
                        nc.scalar.copy(dstc, cps[:])
                    else:
                        nc.vector.tensor_copy(dstc, cps[:])
                    ndrain[0] += 1
                k += 1

        for T in range(NT + 1):
            if T < NT:
                emit_scores(T)
            if T >= 1:
                emit_threshold_masks(T - 1)
                ci, qt = divmod(T - 1, QT)
                mbq[(ci, qt)] = state[T - 1][1]
            emit_colsum(3 if T < NT else None)
        emit_colsum(None)
        nc.sync.dma_start(cnt_d.ap(), out_sb[:])

    nc.compile()
    return nc


def _get_nc():
    if "nc" not in _cache:
        _cache["nc"] = _build_bass()
    return _cache["nc"]


def _kabsch_recon(input_t, sf_t):
    """Mirror reference's f32 Kabsch pipeline in numpy; returns rigid_recon [N,3]."""
    pc = np.ascontiguousarray(input_t[0].T.astype(np.float32))  # [N,3]
    recon = pc + np.ascontiguousarray(sf_t[0].T.astype(np.float32))
    cp = pc.mean(axis=0)
    cr = recon.mean(axis=0)
    H = (pc - cp).T @ (recon - cr)
    U, _, Vt = np.linalg.svd(H.astype(np.float64))
    d = np.sign(np.linalg.det(Vt.T @ U.T))
    R = Vt.T @ (np.array([1.0, 1.0, d])[:, None] * U.T)
    t = cr.astype(np.float64) - R @ cp.astype(np.float64)
    return (pc.astype(np.float64) @ R.T + t).astype(np.float32)


def _split8(v, terms, f8np):
    out = []
    res = v.astype(np.float32)
    for _ in range(terms):
        h = res.astype(f8np)
        out.append(h)
        res = (res - h.astype(np.float32)).astype(np.float32)
    return out


def kernel(input_t, sf_t, y1, pred):
    input_t = np.asarray(input_t, dtype=np.float32)
    sf_t = np.asarray(sf_t, dtype=np.float32)
    y1 = np.asarray(y1, dtype=np.float32)
    pred = np.asarray(pred, dtype=np.float32)

    X = _kabsch_recon(input_t, sf_t)                       # rigid_recon [N,3]
    Y = np.ascontiguousarray(y1[0].T.astype(np.float32))   # [N,3]

    f8np = mybir.dt.np(mybir.dt.float8e4)

    def _pack_ref(R):
        # rows r=0..13 -> [7, 2, N]: (p, i) = (r//2, r%2)
        R2 = (2.0 * R).astype(np.float32)                  # [N,3]
        r1, r2 = _split8(R2, 2, f8np)                      # [N,3] fp8 each
        nr = (R.astype(np.float32) ** 2).sum(axis=1, dtype=np.float32)
        n1, n2 = _split8(nr, 2, f8np)
        rows = [r1.T[d] for d in range(3)] + [r2.T[d] for d in range(3)]
        rows += [r1.T[d] for d in range(3)] + [r2.T[d] for d in range(3)]
        rows += [-n1, -n2]
        return np.ascontiguousarray(np.stack(rows).astype(f8np))

    rx = _pack_ref(X)
    ry = _pack_ref(Y)

    in_maps = []
    one = np.ones(NQ, np.float32).astype(f8np)
    for c in range(NCORES):
        q = X[c * NQ : (c + 1) * NQ].astype(np.float32)    # [NQ,3]
        q1, q2 = _split8(q, 2, f8np)
        rows = [q1.T[d] for d in range(3)] * 2 + [q2.T[d] for d in range(3)] * 2
        rows += [one, one]
        qa = np.ascontiguousarray(np.stack(rows).astype(f8np))
        in_maps.append({"qa": qa, "rx": rx, "ry": ry})

    nc = _get_nc()
    global last_results
    res = run_bass_kernel_spmd(nc, in_maps, core_ids=list(range(NCORES)))
    last_results = res

    cnt = np.stack([r["cnt"].reshape(2, N) for r in res.results]).astype(np.float64)
    cx = cnt[:, 0, :].sum(axis=0)
    cy = cnt[:, 1, :].sum(axis=0)

    Sx = X.astype(np.float64).T @ cx                       # [3]
    Sy = Y.astype(np.float64).T @ cy
    mean_vec = ((Sx - Sy) / ((L_K - 1) * N)).astype(np.float32)

    rigid_refine = X - mean_vec[None, :]
    predT = np.ascontiguousarray(pred[0].T.astype(np.float32))
    loss = np.abs(rigid_refine.astype(np.float64) - predT.astype(np.float64)).mean()
    return np.float32(loss)


# revision 30
# speedup vs baseline: 1.0169x; 1.0169x over previous
"""Trainium2 Bass kernel for nn_ConsistLoss (retrieval_knn).

Math notes
----------
reference() = mean(|rigid_refine - pred^T|) where
  rigid_refine = rigid_recon - mean_i(laplace_x_i - laplace_y_i)
  laplace_c_i  = (sum_{j in 6NN_c(i)} c_j - 6*q_i) / 5       (c in {x=rigid_recon, y})
The -6*q_i terms cancel in (laplace_x - laplace_y), and only the MEAN over all
i is needed, so:
  mean_vec = ( sum_j cx(j)*x_j - sum_j cy(j)*y_j ) / (5*N)
where cx(j) = #queries having ref j among their 6 nearest (mask column sums).

Device work per core (512 queries x 4096 refs x 2 clouds):
  s[q,j] = 2*q.x_j - |x_j|^2  (row-constant |q|^2 shift is rank-safe), via
  fp8e4m3 DoubleRow matmuls: 14 split-product rows in 7 partition pairs,
  0.5 cyc/col.  PSUM f32 -> SBUF fp16 copies (ACT 3 / DVE 1 per query tile),
  threshold via pairwise tensor-max folds 4096->512 (fold1 DVE, fold2/3
  GpSimd) + MAX8.  Masks: refs [0,3072) is_ge(s16,t6) -> bf16 on DVE (4x);
  refs [3072,4096) Sign(s16-mid) -> fp8 +-1 on ACT.  Column sums on PE
  incrementally per query tile (bf16 ones-matmul; fp8 DoubleRow contracts 2
  query tiles per matmul), accumulated in PSUM at 4 partition offsets so one
  [4,512] copy drains 4 chunk-sums.  Host: Kabsch (3x3 SVD) + O(N) decode.
"""

import os
from contextlib import ExitStack

import numpy as np

import concourse.bass as bass  # noqa: F401  (AP types / plumbing)
import concourse.tile as tile
from concourse import bacc, mybir
from concourse.bass_utils import run_bass_kernel_spmd

N = 4096          # points per cloud
NCORES = 8
NQ = N // NCORES  # 512 queries per core
P = 128           # SBUF partitions
QT = NQ // P      # 4 query tiles per core
CHS = 512         # fp32 PSUM bank = 512 f32
KP = 7            # fp8 split rows: 14 = 7 partitions x 2 DoubleRow pairs
NBF = 2048        # refs [0, NBF) -> bf16 masks, rest fp8 (DR colsum)
L_K = 6

_cache = {}
last_results = None  # test harness reads exec_time_ns off this


def _build_bass():
    nc = bacc.Bacc(
        "TRN2", target_bir_lowering=False, debug=False, num_devices=NCORES
    )
    f32 = mybir.dt.float32
    f16 = mybir.dt.float16
    bf16 = mybir.dt.bfloat16
    fp8 = mybir.dt.float8e4
    DR = mybir.MatmulPerfMode.DoubleRow
    NF8 = N - NBF

    qa_d = nc.dram_tensor("qa", [P, NQ], fp8, kind="ExternalInput")
    rx_d = nc.dram_tensor("rx", [P, N], fp8, kind="ExternalInput")
    ry_d = nc.dram_tensor("ry", [P, N], fp8, kind="ExternalInput")
    cnt_d = nc.dram_tensor("cnt", [1, 2 * N], f32, kind="ExternalOutput")

    with ExitStack() as ctx:
        tc = ctx.enter_context(tile.TileContext(nc))
        const_pool = ctx.enter_context(tc.tile_pool(name="const", bufs=1))
        s_pool = ctx.enter_context(tc.tile_pool(name="s16", bufs=3))
        f_pool = ctx.enter_context(tc.tile_pool(name="fold", bufs=3))
        t_pool = ctx.enter_context(tc.tile_pool(name="t8", bufs=3))
        mb_pool = ctx.enter_context(tc.tile_pool(name="mb", bufs=6))
        mf_pool = ctx.enter_context(tc.tile_pool(name="mf", bufs=4))
        ps_pool = ctx.enter_context(tc.tile_pool(name="ps", bufs=2, space="PSUM"))
        cp_pool = ctx.enter_context(tc.tile_pool(name="cp", bufs=2, space="PSUM"))

        qa = const_pool.tile([P, NQ], fp8)
        nc.sync.dma_start(qa[:], qa_d.ap())
        rx = const_pool.tile([P, N], fp8)
        nc.sync.dma_start(rx[:], rx_d.ap())
        ry = const_pool.tile([P, N], fp8)
        nc.sync.dma_start(ry[:], ry_d.ap())
        # DoubleRow ldweights needs the pair-dim step 16B-aligned -> pad
        ones8 = const_pool.tile([P, 2, 16], fp8)
        nc.vector.memset(ones8[:], 1.0)
        onesb = const_pool.tile([P, 1], bf16)
        nc.vector.memset(onesb[:], 1.0)
        out_sb = const_pool.tile([1, 2 * N], f32)

        # --- software-pipelined main loop ------------------------------
        # iteration T: PE scores(T), ACT copies(T); DVE threshold+masks(T-1);
        # PE colsum chunks as clouds complete.  Global tile index T = 4*ci+qt.
        NT = 2 * QT
        state = {}      # T -> (s16, mb, mf, t8)
        mfs = {}        # ci -> [pair0, pair1]
        pend = []       # colsum chunk-sum work items (ci, ch)
        ndrain = [0]

        def emit_scores(T):
            ci, qt = divmod(T, QT)
            r = rx if ci == 0 else ry  # [2*KP, N] fp8
            s16 = s_pool.tile([P, N], f16, tag="s16", name=f"s16_{T}")
            # 2 rounds of 4 row-tiled concurrent matmuls (K=14 per 32-row
            # group; score rows replicated at partitions 0/32/64/96)
            for rd in range(2):
                pss = [
                    ps_pool.tile([P, 2 * CHS], f32, tag="ps", name=f"ps{T}_{rd}_{u}")
                    for u in range(2)
                ]
                for i in range(4):
                    b = 32 * i
                    nc.tensor.matmul(
                        pss[i // 2][:, (i % 2) * CHS : (i % 2 + 1) * CHS],
                        qa[b : b + 2 * KP, qt * P : (qt + 1) * P],
                        r[b : b + 2 * KP, (4 * rd + i) * CHS : (4 * rd + i + 1) * CHS],
                        start=True,
                        stop=True,
                        tile_position=(b, 0),
                    )
                for u in range(2):
                    dst = s16[:, (4 * rd + 2 * u) * CHS : (4 * rd + 2 * u + 2) * CHS]
                    if rd == 1 and u == 1:
                        nc.vector.tensor_copy(dst, pss[u][:])
                    else:
                        nc.scalar.copy(dst, pss[u][:])
            state[T] = s16

        def emit_threshold_masks(T):
            ci, qt = divmod(T, QT)
            s16 = state[T]
            f1 = f_pool.tile([P, N // 2], f16, tag="f1", name=f"f1_{T}")
            nc.vector.tensor_max(f1[:], s16[:, : N // 2], s16[:, N // 2 :])
            f2 = f_pool.tile([P, N // 4], f16, tag="f2", name=f"f2_{T}")
            nc.vector.tensor_max(f2[:], f1[:, : N // 4], f1[:, N // 4 :])
            f3 = f_pool.tile([P, N // 8], f16, tag="f3", name=f"f3_{T}")
            nc.vector.tensor_max(f3[:], f2[:, : N // 8], f2[:, N // 8 :])
            t8 = t_pool.tile([P, 8], f32, tag="t8", name=f"t8_{T}")
            nc.vector.max(t8[:], f3[:])
            # masks vs t6 (exact top-6 incl. fp16 ties):
            # refs [0, NBF): 0/1 bf16 (4x); refs [NBF, N): 0/1 fp8 (2x)
            mb = mb_pool.tile([P, NBF], bf16, tag="mb", name=f"mb_{T}")
            nc.vector.tensor_scalar(
                mb[:], s16[:, :NBF], t8[:, 5:6], None, mybir.AluOpType.is_ge
            )
            if qt % 2 == 0:
                mfs.setdefault(ci, []).append(
                    mf_pool.tile([P, 2, NF8], fp8, tag="mf", name=f"mf_{T}")
                )
            mf = mfs[ci][qt // 2]
            qi = qt % 2
            nc.vector.tensor_scalar(
                mf[:, qi : qi + 1, :],
                s16[:, NBF:N],
                t8[:, 5:6],
                None,
                mybir.AluOpType.is_ge,
            )
            state[T] = (s16, mb)
            if qt == QT - 1:
                pend.extend((ci, ch) for ch in range(N // CHS))

        mbq = {}  # (ci, qt) -> mb tile

        def emit_colsum(budget):
            # emit up to `budget` chunk-sums; drain cp tiles pairwise
            k = 0
            while pend and (budget is None or k < budget):
                ci, ch = pend.pop(0)
                t16 = ci * 8 + ch
                sub = t16 % 2
                if sub == 0:
                    cps = cp_pool.tile(
                        [1, 2 * CHS], f32, tag="cp", name=f"cp_{t16}"
                    )
                    state[("cp", ci, ch // 2)] = cps
                else:
                    cps = state[("cp", ci, ch // 2)]
                dst = cps[:, sub * CHS : (sub + 1) * CHS]
                if ch < NBF // CHS:
                    for qt in range(QT):
                        nc.tensor.matmul(
                            dst,
                            onesb[:],
                            mbq[(ci, qt)][:, ch * CHS : (ch + 1) * CHS],
                            start=(qt == 0),
                            stop=(qt == QT - 1),
                        )
                else:
                    chf = ch - NBF // CHS
                    for pi in range(2):
                        nc.tensor.matmul(
                            dst,
                            ones8[:, :, 0:1],
                            mfs[ci][pi][:, :, chf * CHS : (chf + 1) * CHS],
                            start=(pi == 0),
                            stop=(pi == 1),
                            perf_mode=DR,
                        )
                if sub == 1:
                    col = ci * N + (ch - 1) * CHS
                    dstc = out_sb[0:1, col : col + 2 * CHS]
                    if ndrain[0] % 2 == 0:
                        nc.scalar.copy(dstc, cps[:])
                    else:
                        nc.vector.tensor_copy(dstc, cps[:])
                    ndrain[0] += 1
                k += 1

        for T in range(NT + 1):
            if T < NT:
                emit_scores(T)
            if T >= 1:
                emit_threshold_masks(T - 1)
                ci, qt = divmod(T - 1, QT)
                mbq[(ci, qt)] = state[T - 1][1]
            emit_colsum(3 if T < NT else None)
        emit_colsum(None)
        nc.sync.dma_start(cnt_d.ap(), out_sb[:])

    nc.compile()
    return nc


def _get_nc():
    if "nc" not in _cache:
        _cache["nc"] = _build_bass()
    return _cache["nc"]


def _kabsch_recon(input_t, sf_t):
    """Mirror reference's f32 Kabsch pipeline in numpy; returns rigid_recon [N,3]."""
    pc = np.ascontiguousarray(input_t[0].T.astype(np.float32))  # [N,3]
    recon = pc + np.ascontiguousarray(sf_t[0].T.astype(np.float32))
    cp = pc.mean(axis=0)
    cr = recon.mean(axis=0)
    H = (pc - cp).T @ (recon - cr)
    U, _, Vt = np.linalg.svd(H.astype(np.float64))
    d = np.sign(np.linalg.det(Vt.T @ U.T))
    R = Vt.T @ (np.array([1.0, 1.0, d])[:, None] * U.T)
    t = cr.astype(np.float64) - R @ cp.astype(np.float64)
    return (pc.astype(np.float64) @ R.T + t).astype(np.float32)


def _split8(v, terms, f8np):
    out = []
    res = v.astype(np.float32)
    for _ in range(terms):
        h = res.astype(f8np)
        out.append(h)
        res = (res - h.astype(np.float32)).astype(np.float32)
    return out


def kernel(input_t, sf_t, y1, pred):
    input_t = np.asarray(input_t, dtype=np.float32)
    sf_t = np.asarray(sf_t, dtype=np.float32)
    y1 = np.asarray(y1, dtype=np.float32)
    pred = np.asarray(pred, dtype=np.float32)

    X = _kabsch_recon(input_t, sf_t)                       # rigid_recon [N,3]
    Y = np.ascontiguousarray(y1[0].T.astype(np.float32))   # [N,3]

    f8np = mybir.dt.np(mybir.dt.float8e4)

    def _pack_ref(R):
        # rows r=0..13 -> [7, 2, N]: (p, i) = (r//2, r%2)
        R2 = (2.0 * R).astype(np.float32)                  # [N,3]
        r1, r2 = _split8(R2, 2, f8np)                      # [N,3] fp8 each
        nr = (R.astype(np.float32) ** 2).sum(axis=1, dtype=np.float32)
        n1, n2 = _split8(nr, 2, f8np)
        rows = [r1.T[d] for d in range(3)] + [r2.T[d] for d in range(3)]
        rows += [r1.T[d] for d in range(3)] + [r2.T[d] for d in range(3)]
        rows += [-n1, -n2]
        blk = np.stack(rows).astype(f8np)                  # [14, N]
        out = np.zeros((P, blk.shape[1]), f8np)
        for i in range(4):
            out[32 * i : 32 * i + 2 * KP] = blk
        return np.ascontiguousarray(out)

    rx = _pack_ref(X)
    ry = _pack_ref(Y)

    in_maps = []
    one = np.ones(NQ, np.float32).astype(f8np)
    for c in range(NCORES):
        q = X[c * NQ : (c + 1) * NQ].astype(np.float32)    # [NQ,3]
        q1, q2 = _split8(q, 2, f8np)
        rows = [q1.T[d] for d in range(3)] * 2 + [q2.T[d] for d in range(3)] * 2
        rows += [one, one]
        blk = np.stack(rows).astype(f8np)
        qa = np.zeros((P, NQ), f8np)
        for i in range(4):
            qa[32 * i : 32 * i + 2 * KP] = blk
        qa = np.ascontiguousarray(qa)
        in_maps.append({"qa": qa, "rx": rx, "ry": ry})

    nc = _get_nc()
    global last_results
    res = run_bass_kernel_spmd(nc, in_maps, core_ids=list(range(NCORES)))
    last_results = res

    cnt = np.stack([r["cnt"].reshape(2, N) for r in res.results]).astype(np.float64)
    cx = cnt[:, 0, :].sum(axis=0)
    cy = cnt[:, 1, :].sum(axis=0)

    Sx = X.astype(np.float64).T @ cx                       # [3]
    Sy = Y.astype(np.float64).T @ cy
    mean_vec = ((Sx - Sy) / ((L_K - 1) * N)).astype(np.float32)

    rigid_refine = X - mean_vec[None, :]
    predT = np.ascontiguousarray(pred[0].T.astype(np.float32))
    loss = np.abs(rigid_refine.astype(np.float64) - predT.astype(np.float64)).mean()
    return np.float32(loss)


# revision 36
# speedup vs baseline: 1.1953x; 1.1755x over previous
"""Trainium2 Bass kernel for nn_ConsistLoss (retrieval_knn).

Math notes
----------
reference() = mean(|rigid_refine - pred^T|) where
  rigid_refine = rigid_recon - mean_i(laplace_x_i - laplace_y_i)
  laplace_c_i  = (sum_{j in 6NN_c(i)} c_j - 6*q_i) / 5       (c in {x=rigid_recon, y})
The -6*q_i terms cancel in (laplace_x - laplace_y), and only the MEAN over all
i is needed, so:
  mean_vec = ( sum_j cx(j)*x_j - sum_j cy(j)*y_j ) / (5*N)
where cx(j) = #queries having ref j among their 6 nearest (mask column sums).

Device work per core (512 queries x 4096 refs x 2 clouds):
  s[q,j] = 2*q.x_j - |x_j|^2  (row-constant |q|^2 shift is rank-safe), via
  fp8e4m3 DoubleRow matmuls: 14 split-product rows in 7 partition pairs,
  0.5 cyc/col.  PSUM f32 -> SBUF fp16 copies (ACT 3 / DVE 1 per query tile),
  threshold via pairwise tensor-max folds 4096->512 (fold1 DVE, fold2/3
  GpSimd) + MAX8.  Masks: refs [0,3072) is_ge(s16,t6) -> bf16 on DVE (4x);
  refs [3072,4096) Sign(s16-mid) -> fp8 +-1 on ACT.  Column sums on PE
  incrementally per query tile (bf16 ones-matmul; fp8 DoubleRow contracts 2
  query tiles per matmul), accumulated in PSUM at 4 partition offsets so one
  [4,512] copy drains 4 chunk-sums.  Host: Kabsch (3x3 SVD) + O(N) decode.
"""

import os
from contextlib import ExitStack

import numpy as np

import concourse.bass as bass  # noqa: F401  (AP types / plumbing)
import concourse.tile as tile
from concourse import bacc, mybir
from concourse.bass_utils import run_bass_kernel_spmd

N = 4096          # points per cloud
NCORES = 8
NQ = N // NCORES  # 512 queries per core
P = 128           # SBUF partitions
QT = NQ // P      # 4 query tiles per core
CHS = 512         # fp32 PSUM bank = 512 f32
KP = 7            # fp8 split rows: 14 = 7 partitions x 2 DoubleRow pairs
L_K = 6

_cache = {}
last_results = None  # test harness reads exec_time_ns off this


def _build_bass():
    nc = bacc.Bacc(
        "TRN2", target_bir_lowering=False, debug=False, num_devices=NCORES
    )
    f32 = mybir.dt.float32
    f16 = mybir.dt.float16
    bf16 = mybir.dt.bfloat16
    fp8 = mybir.dt.float8e4

    qa_d = nc.dram_tensor("qa", [P, NQ], fp8, kind="ExternalInput")
    rx_d = nc.dram_tensor("rx", [P, N], fp8, kind="ExternalInput")
    ry_d = nc.dram_tensor("ry", [P, N], fp8, kind="ExternalInput")
    cnt_d = nc.dram_tensor("cnt", [4, 2, 4 * CHS // 2], f32, kind="ExternalOutput")

    with ExitStack() as ctx:
        tc = ctx.enter_context(tile.TileContext(nc))
        const_pool = ctx.enter_context(tc.tile_pool(name="const", bufs=1))
        s_pool = ctx.enter_context(tc.tile_pool(name="s16", bufs=3))
        f_pool = ctx.enter_context(tc.tile_pool(name="fold", bufs=3))
        t_pool = ctx.enter_context(tc.tile_pool(name="t8", bufs=3))
        mb_pool = ctx.enter_context(tc.tile_pool(name="mb", bufs=6))
        ps_pool = ctx.enter_context(tc.tile_pool(name="ps", bufs=2, space="PSUM"))
        cp_pool = ctx.enter_context(tc.tile_pool(name="cp", bufs=4, space="PSUM"))

        qa = const_pool.tile([P, NQ], fp8)
        nc.sync.dma_start(qa[:], qa_d.ap())
        rx = const_pool.tile([P, N], fp8)
        nc.sync.dma_start(rx[:], rx_d.ap())
        ry = const_pool.tile([P, N], fp8)
        nc.sync.dma_start(ry[:], ry_d.ap())
        onesb32 = const_pool.tile([P, 32], bf16)
        nc.vector.memset(onesb32[:], 1.0)
        out_sb = const_pool.tile([P, 2, 2 * CHS], f32)

        # --- software-pipelined main loop ------------------------------
        # iteration T: PE scores(T), ACT copies(T); DVE threshold+masks(T-1);
        # PE colsum chunks as clouds complete.  Global tile index T = 4*ci+qt.
        NT = 2 * QT
        state = {}      # T -> s16

        def emit_scores(T):
            ci, qt = divmod(T, QT)
            r = rx if ci == 0 else ry  # [2*KP, N] fp8
            s16 = s_pool.tile([P, N], f16, tag="s16", name=f"s16_{T}")
            # 2 rounds of 4 row-tiled concurrent matmuls (K=14 per 32-row
            # group; score rows replicated at partitions 0/32/64/96)
            for rd in range(2):
                pss = [
                    ps_pool.tile([P, 2 * CHS], f32, tag="ps", name=f"ps{T}_{rd}_{u}")
                    for u in range(2)
                ]
                for i in range(4):
                    b = 32 * i
                    nc.tensor.matmul(
                        pss[i // 2][:, (i % 2) * CHS : (i % 2 + 1) * CHS],
                        qa[b : b + 2 * KP, qt * P : (qt + 1) * P],
                        r[b : b + 2 * KP, (4 * rd + i) * CHS : (4 * rd + i + 1) * CHS],
                        start=True,
                        stop=True,
                        tile_position=(b, 0),
                    )
                for u in range(2):
                    dst = s16[:, (4 * rd + 2 * u) * CHS : (4 * rd + 2 * u + 2) * CHS]
                    nc.scalar.copy(dst, pss[u][:])
            state[T] = s16

        def emit_threshold_masks(T):
            ci, qt = divmod(T, QT)
            s16 = state[T]
            f1 = f_pool.tile([P, N // 2], f16, tag="f1", name=f"f1_{T}")
            nc.vector.tensor_max(f1[:], s16[:, : N // 2], s16[:, N // 2 :])
            f2 = f_pool.tile([P, N // 4], f16, tag="f2", name=f"f2_{T}")
            nc.vector.tensor_max(f2[:], f1[:, : N // 4], f1[:, N // 4 :])
            f3 = f_pool.tile([P, N // 8], f16, tag="f3", name=f"f3_{T}")
            nc.vector.tensor_max(f3[:], f2[:, : N // 8], f2[:, N // 8 :])
            t8 = t_pool.tile([P, 8], f32, tag="t8", name=f"t8_{T}")
            nc.vector.max(t8[:], f3[:])
            # mask vs t6 (exact top-6 incl. fp16 ties): 0/1 bf16, 4x DVE
            mb = mb_pool.tile([P, N], bf16, tag="mb", name=f"mb_{T}")
            nc.vector.tensor_scalar(
                mb[:], s16[:], t8[:, 5:6], None, mybir.AluOpType.is_ge
            )
            return mb

        # column-sum layout: chunk c of cloud ci -> cp bank c%4, col-group
        # CG[c] (output partitions 32*CG[c]); per bank the two chunks occupy
        # 64 contiguous rows, drained by one [64, 512] copy.  Chunks 0-3
        # accumulate incrementally per query tile (one open group per bank);
        # chunks 4-7 run as a burst after the cloud's last mask (their bank's
        # first group is closed by then).
        CG = [0, 1, 2, 3, 1, 0, 3, 2]
        BROW = [0, 0, 64, 64]  # drain start row per bank

        def emit_colsum_leg(ci, qt, mb):
            for c in range(4):
                cp = cps[(ci, c)]
                pp = 32 * CG[c]
                nc.tensor.matmul(
                    cp[pp : pp + 32, :],
                    onesb32[:],
                    mb[:, c * CHS : (c + 1) * CHS],
                    start=(qt == 0),
                    stop=(qt == QT - 1),
                    tile_position=(0, pp),
                )

        def emit_colsum_tail(ci):
            for qt in range(QT):
                for c in range(4, 8):
                    cp = cps[(ci, c % 4)]
                    pp = 32 * CG[c]
                    nc.tensor.matmul(
                        cp[pp : pp + 32, :],
                        onesb32[:],
                        mbq[(ci, qt)][:, c * CHS : (c + 1) * CHS],
                        start=(qt == 0),
                        stop=(qt == QT - 1),
                        tile_position=(0, pp),
                    )

        def emit_drains(ci):
            for b in range(4):
                cp = cps[(ci, b)]
                r0 = BROW[b]
                bb = (b % 2) * CHS
                dstc = out_sb[r0 : r0 + 64, ci : ci + 1, bb : bb + CHS]
                nc.vector.tensor_copy(dstc, cp[r0 : r0 + 64, :])

        cps = {}
        mbq = {}
        for T in range(NT + 1):
            if T < NT:
                ci = T // QT
                if T % QT == 0:
                    for b in range(4):
                        cps[(ci, b)] = cp_pool.tile(
                            [P, CHS], f32, tag="cp", name=f"cp{ci}_{b}"
                        )
                emit_scores(T)
            if T >= 1:
                ci, qt = divmod(T - 1, QT)
                mb = emit_threshold_masks(T - 1)
                mbq[(ci, qt)] = mb
                emit_colsum_leg(ci, qt, mb)
                if qt == QT - 1:
                    emit_colsum_tail(ci)
                    emit_drains(ci)
        # rows {0,32}: banks 0,1 (inner cols 0:1024); rows {64,96}: banks 2,3
        for g in range(4):
            nc.sync.dma_start(
                cnt_d.ap()[g : g + 1, :, :],
                out_sb[32 * g : 32 * g + 1, :, :],
            )

    nc.compile()
    return nc


def _get_nc():
    if "nc" not in _cache:
        _cache["nc"] = _build_bass()
    return _cache["nc"]


def _kabsch_recon(input_t, sf_t):
    """Mirror reference's f32 Kabsch pipeline in numpy; returns rigid_recon [N,3]."""
    pc = np.ascontiguousarray(input_t[0].T.astype(np.float32))  # [N,3]
    recon = pc + np.ascontiguousarray(sf_t[0].T.astype(np.float32))
    cp = pc.mean(axis=0)
    cr = recon.mean(axis=0)
    H = (pc - cp).T @ (recon - cr)
    U, _, Vt = np.linalg.svd(H.astype(np.float64))
    d = np.sign(np.linalg.det(Vt.T @ U.T))
    R = Vt.T @ (np.array([1.0, 1.0, d])[:, None] * U.T)
    t = cr.astype(np.float64) - R @ cp.astype(np.float64)
    return (pc.astype(np.float64) @ R.T + t).astype(np.float32)


def _split8(v, terms, f8np):
    out = []
    res = v.astype(np.float32)
    for _ in range(terms):
        h = res.astype(f8np)
        out.append(h)
        res = (res - h.astype(np.float32)).astype(np.float32)
    return out


def kernel(input_t, sf_t, y1, pred):
    input_t = np.asarray(input_t, dtype=np.float32)
    sf_t = np.asarray(sf_t, dtype=np.float32)
    y1 = np.asarray(y1, dtype=np.float32)
    pred = np.asarray(pred, dtype=np.float32)

    X = _kabsch_recon(input_t, sf_t)                       # rigid_recon [N,3]
    Y = np.ascontiguousarray(y1[0].T.astype(np.float32))   # [N,3]

    f8np = mybir.dt.np(mybir.dt.float8e4)

    def _pack_ref(R):
        # rows r=0..13 -> [7, 2, N]: (p, i) = (r//2, r%2)
        R2 = (2.0 * R).astype(np.float32)                  # [N,3]
        r1, r2 = _split8(R2, 2, f8np)                      # [N,3] fp8 each
        nr = (R.astype(np.float32) ** 2).sum(axis=1, dtype=np.float32)
        n1, n2 = _split8(nr, 2, f8np)
        rows = [r1.T[d] for d in range(3)] + [r2.T[d] for d in range(3)]
        rows += [r1.T[d] for d in range(3)] + [r2.T[d] for d in range(3)]
        rows += [-n1, -n2]
        blk = np.stack(rows).astype(f8np)                  # [14, N]
        out = np.zeros((P, blk.shape[1]), f8np)
        for i in range(4):
            out[32 * i : 32 * i + 2 * KP] = blk
        return np.ascontiguousarray(out)

    rx = _pack_ref(X)
    ry = _pack_ref(Y)

    in_maps = []
    one = np.ones(NQ, np.float32).astype(f8np)
    for c in range(NCORES):
        q = X[c * NQ : (c + 1) * NQ].astype(np.float32)    # [NQ,3]
        q1, q2 = _split8(q, 2, f8np)
        rows = [q1.T[d] for d in range(3)] * 2 + [q2.T[d] for d in range(3)] * 2
        rows += [one, one]
        blk = np.stack(rows).astype(f8np)
        qa = np.zeros((P, NQ), f8np)
        for i in range(4):
            qa[32 * i : 32 * i + 2 * KP] = blk
        qa = np.ascontiguousarray(qa)
        in_maps.append({"qa": qa, "rx": rx, "ry": ry})

    nc = _get_nc()
    global last_results
    res = run_bass_kernel_spmd(nc, in_maps, core_ids=list(range(NCORES)))
    last_results = res

    CG = [0, 1, 2, 3, 1, 0, 3, 2]
    cnt = np.zeros((NCORES, 2, N), np.float64)
    for core in range(NCORES):
        raw = res.results[core]["cnt"].astype(np.float64).reshape(4, 2, 1024)
        for ci in range(2):
            for c in range(8):
                cnt[core, ci, c * CHS : (c + 1) * CHS] = raw[
                    CG[c], ci, (c % 2) * CHS : (c % 2 + 1) * CHS
                ]
    cx = cnt[:, 0, :].sum(axis=0)
    cy = cnt[:, 1, :].sum(axis=0)

    Sx = X.astype(np.float64).T @ cx                       # [3]
    Sy = Y.astype(np.float64).T @ cy
    mean_vec = ((Sx - Sy) / ((L_K - 1) * N)).astype(np.float32)

    rigid_refine = X - mean_vec[None, :]
    predT = np.ascontiguousarray(pred[0].T.astype(np.float32))
    loss = np.abs(rigid_refine.astype(np.float64) - predT.astype(np.float64)).mean()
    return np.float32(loss)
